# revision 2
# baseline (speedup 1.0000x reference)
"""Trainium2 Bass kernel for causal multi-head attention with interleaved RoPE.

Problem: B=2, S=2048, D=1024, 16 heads x 64 dims, causal, rope theta=1e4.

Sharding (8 cores): 2-way batch x 4-way head tensor-parallel.
  core i: batch b = i // 4, head group g = i % 4 (heads 4g..4g+3, dims 256).
  Each core computes q/k/v for its heads from x[b], runs causal flash
  attention, and produces a partial output projection outT [D, S].  Host
  sums the 4 partials per batch and transposes.

Performance design (v3 — fused single pipeline):
  The three phases (qkv projection, attention, output projection) are
  software-pipelined per 512-query tile instead of running serially:
  while the scalar engine streams the softmax EXPs of q-tile qt (the hard
  ~60us floor of this kernel), the PE runs the qkv projection of tile
  qt+1 and the output projection of tile qt-1, interleaved between score/
  AV chunk matmuls in emission order.  PSUM (8 banks) is split: scores /
  output-proj / recip-broadcast share one 2-buf x 2-bank ring ("s"),
  q/k/v projection shares a 2-buf x 1-bank ring ("qk"), and the two AV
  accumulators take 1 bank each.

  Carried over from v2:
  - Inputs in bf16; x streamed in 512-column chunks.
  - Scores: the two heads of a 128-partition group are a row-tiled
    concurrent matmul pair, EXPed by a single scalar activation.
  - AV col-tiled pair into two PSUM banks; ones-row trick (M=65) yields
    softmax denominators in psum row 64.
  - Denominators inverted per-tile with reciprocal_approx_fast and
    broadcast back to 64 dims with a selection matmul.
"""

import os
import sys

sys.path.insert(0, "/opt/trn_rl_repo")

import numpy as np

B = 2
S = 2048
D = 1024
NH = 16
HD = 64
THETA = 10000.0
NCORES = 8
HPC = 4  # heads per core
DC = HPC * HD  # 256 dims per core
GQ = 2  # 128-partition head groups per core
QT = 512  # query tile (free dim)
NQT = S // QT
KC = 128  # key chunk (partition dim)

_CACHE = {}


def _install_axon_ntff_hook():
    """Register antenv.axon_hooks so trace=True (BASS_TRACE=1) works."""
    import types

    if "antenv.axon_hooks" in sys.modules:
        return
    m = types.ModuleType("antenv.axon_hooks")
    _hook = [None]
    m.set_axon_ntff_profile_hook = lambda h: _hook.__setitem__(0, h)
    m.get_axon_ntff_profile_hook = lambda: _hook[0]
    sys.modules["antenv.axon_hooks"] = m
    try:
        import antenv

        antenv.axon_hooks = m
        from trn_agent_boot.trn_boot import _ntff_profile_via_ctypes

        hook = _ntff_profile_via_ctypes("/opt/axon/libaxon_pjrt.so")
        if hook is not None:
            m.set_axon_ntff_profile_hook(hook)
    except Exception:
        pass


def _rope_perm_local():
    """Permutation of one head's 64 dims: original interleaved pair (2i, 2i+1)
    -> t0 at quadrant*32 + (i%16), t1 at quadrant*32 + 16 + (i%16), with
    quadrant = i // 16.  Returns perm such that new[j] = old[perm[j]]."""
    perm = np.zeros(HD, dtype=np.int64)
    for i in range(HD // 2):
        qd, r = divmod(i, 16)
        perm[qd * 32 + r] = 2 * i
        perm[qd * 32 + 16 + r] = 2 * i + 1
    return perm


def _rope_tables():
    """cos_dup/sin_signed [128, S]: per-partition rope tables matching the
    de-interleaved layout (pattern repeats every 64 partitions)."""
    inv_freq = 1.0 / (THETA ** (np.arange(0, HD, 2, dtype=np.float64) / HD))  # [32]
    pos = np.arange(S, dtype=np.float64)
    ang = pos[None, :] * inv_freq[:, None]  # [32, S]
    cos = np.cos(ang)
    sin = np.sin(ang)
    cos_dup = np.zeros((128, S), dtype=np.float32)
    sin_signed = np.zeros((128, S), dtype=np.float32)
    for p in range(128):
        d = p % HD
        qd, r0 = divmod(d, 32)
        if r0 < 16:
            i = qd * 16 + r0
            cos_dup[p] = cos[i]
            sin_signed[p] = -sin[i]
        else:
            i = qd * 16 + (r0 - 16)
            cos_dup[p] = cos[i]
            sin_signed[p] = sin[i]
    return cos_dup, sin_signed


def _build_program():
    import concourse.bass as bass
    from concourse import bacc, mybir
    import concourse.tile as tile

    f32 = mybir.dt.float32
    bf16 = mybir.dt.bfloat16
    ADD = mybir.AluOpType.add
    MULT = mybir.AluOpType.mult
    EXP = mybir.ActivationFunctionType.Exp
    SWAP16 = [(j + 16) % 32 for j in range(32)]
    DK = D // 128  # 8 contraction chunks

    nc = bacc.Bacc("TRN2", target_bir_lowering=False, debug=False)
    # xT pre-arranged on host as [128, D//128, S] so every q-tile chunk is a
    # dense per-partition block (16KB/partition locality for the DMA).
    xT = nc.dram_tensor("xT", [128, (D // 128) * S], bf16,
                        kind="ExternalInput").ap()
    wq = nc.dram_tensor("wq", [D, DC], bf16, kind="ExternalInput").ap()
    wk = nc.dram_tensor("wk", [D, DC], bf16, kind="ExternalInput").ap()
    wv = nc.dram_tensor("wv", [D, DC], bf16, kind="ExternalInput").ap()
    wo = nc.dram_tensor("wo", [DC, D], bf16, kind="ExternalInput").ap()
    cosd = nc.dram_tensor("cosd", [128, S], bf16, kind="ExternalInput").ap()
    sind = nc.dram_tensor("sind", [128, S], bf16, kind="ExternalInput").ap()
    tri = nc.dram_tensor("tri", [KC, KC], bf16, kind="ExternalInput").ap()
    sel = nc.dram_tensor("sel", [128, GQ * 128], bf16,
                         kind="ExternalInput").ap()
    vone = nc.dram_tensor("vone", [128, (S // KC) * HPC], bf16,
                          kind="ExternalInput").ap()
    outT = nc.dram_tensor("outT", [D, S], bf16, kind="ExternalOutput").ap()

    with tile.TileContext(nc) as tc:
        with tc.tile_pool(name="const", bufs=1) as const:
            cos_sb = const.tile([128, S], bf16)
            sin_sb = const.tile([128, S], bf16)
            tri_sb = const.tile([KC, KC], bf16)
            wq_sb = const.tile([128, DK, DC], bf16)
            wk_sb = const.tile([128, DK, DC], bf16)
            wv_sb = const.tile([128, DK, DC], bf16)
            wo_sb = const.tile([128, GQ, D], bf16)
            xT_sb = const.tile([128, DK, S], bf16)
            qT_sb = const.tile([128, GQ, S], bf16)
            kT_sb = const.tile([128, GQ, S], bf16)
            vaug_sb = const.tile([128, S // KC, HPC * (HD + 1)], bf16)
            oT_sb = const.tile([128, GQ, S], bf16)
            sums_sb = const.tile([128, S], f32)
            recip_sb = const.tile([128, S], f32)
            recip_bf = const.tile([128, S], bf16)
            sel_sb = const.tile([128, GQ, 128], bf16)

            # DMA order tuned so the projection of tile 0 starts ~5us in.
            xTr = xT.rearrange("p (o n) -> p o n", o=D // 128)
            nc.sync.dma_start(wq_sb, wq.rearrange("(o p) n -> p o n", p=128))
            nc.sync.dma_start(xT_sb[:, :, 0:QT], xTr[:, :, 0:QT])
            nc.sync.dma_start(wk_sb, wk.rearrange("(o p) n -> p o n", p=128))
            nc.sync.dma_start(cos_sb[:, 0:QT], cosd[:, 0:QT])
            nc.sync.dma_start(sin_sb[:, 0:QT], sind[:, 0:QT])
            nc.sync.dma_start(wv_sb, wv.rearrange("(o p) n -> p o n", p=128))
            nc.sync.dma_start(tri_sb, tri)
            nc.sync.dma_start(
                vaug_sb[:, :, HD::(HD + 1)],
                vone.rearrange("p (a b) -> p a b", a=S // KC))
            nc.sync.dma_start(
                sel_sb, sel.rearrange("p (c n) -> p c n", c=GQ))
            for qt in range(1, NQT):
                q0 = qt * QT
                nc.sync.dma_start(xT_sb[:, :, q0:q0 + QT], xTr[:, :, q0:q0 + QT])
                nc.sync.dma_start(cos_sb[:, q0:q0 + QT], cosd[:, q0:q0 + QT])
                nc.sync.dma_start(sin_sb[:, q0:q0 + QT], sind[:, q0:q0 + QT])
            nc.sync.dma_start(wo_sb, wo.rearrange("(o p) n -> p o n", p=128))

            # ---- single fused pipeline ----
            # PSUM budget (8 banks): pp "qk" 2x1, pss "s" 2x2, po o0/o1 1x1+1x1.
            with tc.tile_pool(name="pp", bufs=2, space="PSUM") as pp, \
                 tc.tile_pool(name="pss", bufs=2, space="PSUM") as pss, \
                 tc.tile_pool(name="po", bufs=1, space="PSUM") as po, \
                 tc.tile_pool(name="tmp1", bufs=3) as tmp1, \
                 tc.tile_pool(name="ppr", bufs=5) as ppr, \
                 tc.tile_pool(name="p5s", bufs=6) as p5s:
                import concourse.bass as _b

                def rope(ps, dst, q0):
                    # pure-bf16 chain after one psum cast: 2x DVE throughput
                    qb = tmp1.tile([128, QT], bf16, tag="qb")
                    nc.vector.tensor_copy(out=qb, in_=ps)
                    shuf = tmp1.tile([128, QT], bf16, tag="shuf")
                    nc.vector.stream_shuffle(shuf, qb, SWAP16)
                    m1 = tmp1.tile([128, QT], bf16, tag="m1")
                    nc.vector.tensor_tensor(m1, qb, cos_sb[:, q0:q0 + QT], MULT)
                    m2 = tmp1.tile([128, QT], bf16, tag="m2")
                    nc.vector.tensor_tensor(m2, shuf, sin_sb[:, q0:q0 + QT], MULT)
                    nc.vector.tensor_tensor(dst, m1, m2, ADD)

                def qk_unit(qt, g, which):
                    def run():
                        q0 = qt * QT
                        w_sb = wq_sb if which == "q" else wk_sb
                        dst = qT_sb if which == "q" else kT_sb
                        ps = pp.tile([128, QT], f32, tag="qk",
                                     name=f"ps_{which}{g}_{qt}")
                        for kc in range(DK):
                            nc.tensor.matmul(
                                ps, w_sb[:, kc, g * 128:(g + 1) * 128],
                                xT_sb[:, kc, q0:q0 + QT],
                                start=(kc == 0), stop=(kc == DK - 1))
                        rope(ps, dst[:, g, q0:q0 + QT], q0)
                    return run

                def v_unit(qt, rc):
                    def run():
                        r0 = qt * QT + rc * KC
                        ps = pp.tile([128, QT], f32, tag="qk",
                                     name=f"ps_v_{qt}_{rc}")
                        psv = ps[:, 0:DC]
                        for kc in range(DK):
                            nc.tensor.matmul(
                                psv, xT_sb[:, kc, r0:r0 + KC],
                                wv_sb[:, kc, :],
                                start=(kc == 0), stop=(kc == DK - 1))
                        # one strided copy: psum [128,(h d)] -> vaug 65-pitch
                        vdst = vaug_sb[:, r0 // KC, 0:HD]
                        dst3 = _b.AP(tensor=vdst.tensor, offset=vdst.offset,
                                     ap=[list(vdst.ap[0]), [HD + 1, HPC],
                                         [1, HD]])
                        src3 = _b.AP(tensor=psv.tensor, offset=psv.offset,
                                     ap=[list(psv.ap[0]), [HD, HPC],
                                         [1, HD]])
                        nc.vector.tensor_copy(out=dst3, in_=src3)
                    return run

                def sums_init_unit(qt):
                    # sums init to 1.0 (memset >1 column miscompiles): garbage
                    # lanes must stay finite-nonzero for the reciprocal.
                    def run():
                        q0 = qt * QT
                        nc.vector.tensor_scalar(
                            sums_sb[:, q0:q0 + QT], cos_sb[:, q0:q0 + QT],
                            0.0, 1.0, MULT, ADD)
                    return run

                def proj_stream(qt):
                    us = [sums_init_unit(qt)]
                    for g in range(GQ):
                        us.append(qk_unit(qt, g, "q"))
                        us.append(qk_unit(qt, g, "k"))
                    for rc in range(QT // KC):
                        us.append(v_unit(qt, rc))
                    return us

                # ---- attention chunk machinery (S^T orientation) ----
                state = {}  # (g, qt) -> ps_o pair
                pendq = []  # [(g, qt, kc, nkc, probs, qlo)]

                def emit_av(p):
                    g, qt, kc, nkc, probs, qlo = p
                    q0 = qt * QT
                    for a in range(2):
                        h = 2 * g + a
                        nc.tensor.matmul(
                            state[(g, qt)][a][:, qlo:QT],
                            vaug_sb[:, kc, h * (HD + 1):(h + 1) * (HD + 1)],
                            probs[:, a, qlo:QT],
                            start=(kc == 0), stop=(kc == nkc - 1))
                    if kc == nkc - 1:
                        for a in range(2):
                            h = 2 * g + a
                            nc.vector.tensor_copy(
                                out=oT_sb[a * HD:(a + 1) * HD, g, q0:q0 + QT],
                                in_=state[(g, qt)][a][0:HD, :])
                            nc.vector.tensor_copy(
                                out=sums_sb[32 * h:32 * h + 1, q0:q0 + QT],
                                in_=state[(g, qt)][a][HD:HD + 1, :])
                        del state[(g, qt)]

                def emit_chunk(g, qt, kc, nkc):
                    q0 = qt * QT
                    k0 = kc * KC
                    j = k0 - q0
                    qlo = max(0, j)
                    if kc == 0:
                        state[(g, qt)] = [
                            po.tile([HD + 1, QT], f32, tag=f"o{a}",
                                    name=f"ps_o{g}_{qt}_{a}")
                            for a in range(2)]
                    ps_s = pss.tile([128, 2, QT], f32, tag="s",
                                    name=f"ps_s{g}_{qt}_{kc}")
                    for a in range(2):
                        nc.tensor.matmul(
                            ps_s[:, a, qlo:QT],
                            kT_sb[a * HD:(a + 1) * HD, g, k0:k0 + KC],
                            qT_sb[a * HD:(a + 1) * HD, g, q0 + qlo:q0 + QT],
                            start=True, stop=True)
                    if len(pendq) >= 2:
                        emit_av(pendq.pop(0))
                    probs = ppr.tile([128, 2, QT], bf16, tag="p")
                    nc.scalar.activation(
                        probs[:, :, qlo:QT], ps_s[:, :, qlo:QT], EXP)
                    if j >= 0:
                        # mask the diag block on the probs (0/1 multiply):
                        # keeps the DVE off the S->EXP critical chain
                        for a in range(2):
                            nc.vector.tensor_tensor(
                                probs[:, a, qlo:qlo + KC],
                                probs[:, a, qlo:qlo + KC], tri_sb, MULT)
                    pendq.append((g, qt, kc, nkc, probs, qlo))

                # ---- normalize + output projection units ----
                def recip_unit(qt):
                    def run():
                        q0 = qt * QT
                        nc.vector.reciprocal_approx_fast(
                            recip_sb[:, q0:q0 + QT], sums_sb[:, q0:q0 + QT])
                        nc.vector.tensor_copy(
                            out=recip_bf[:, q0:q0 + QT],
                            in_=recip_sb[:, q0:q0 + QT])
                    return run

                def norm_unit(qt, g):
                    def run():
                        q0 = qt * QT
                        pr = pss.tile([128, 2, QT], f32, tag="s",
                                      name=f"pr_{qt}_{g}")
                        nc.tensor.matmul(pr[:, 0, :], sel_sb[:, g, :],
                                         recip_bf[:, q0:q0 + QT],
                                         start=True, stop=True)
                        nc.vector.tensor_tensor(
                            oT_sb[:, g, q0:q0 + QT], oT_sb[:, g, q0:q0 + QT],
                            pr[:, 0, :], MULT)
                    return run

                def outproj_unit(qt, ec):
                    def run():
                        q0 = qt * QT
                        ps = pss.tile([128, 2, QT], f32, tag="s",
                                      name=f"ps_f_{qt}_{ec}")
                        for g in range(GQ):
                            nc.tensor.matmul(
                                ps[:, 0, :],
                                wo_sb[:, g, ec * 128:(ec + 1) * 128],
                                oT_sb[:, g, q0:q0 + QT],
                                start=(g == 0), stop=(g == GQ - 1))
                        ob = p5s.tile([128, QT], bf16, tag="ob")
                        nc.vector.tensor_copy(out=ob, in_=ps[:, 0, :])
                        nc.sync.dma_start(
                            outT[ec * 128:(ec + 1) * 128, q0:q0 + QT], ob)
                    return run

                def out_stream(qt):
                    us = [recip_unit(qt), norm_unit(qt, 0), norm_unit(qt, 1)]
                    us += [outproj_unit(qt, ec) for ec in range(D // 128)]
                    return us

                # ---- master schedule ----
                for u in proj_stream(0):
                    u()

                for qt in range(NQT):
                    projs = proj_stream(qt + 1) if qt + 1 < NQT else []
                    outs = out_stream(qt - 1) if qt >= 1 else []
                    np_, no_ = len(projs), len(outs)
                    nkc = 4 * (qt + 1)
                    chunks = [(g, kc) for g in range(GQ) for kc in range(nkc)]
                    nch = len(chunks)
                    nin = np_ + no_
                    pi = oi = 0
                    for ci, (g, kc) in enumerate(chunks):
                        emit_chunk(g, qt, kc, nkc)
                        target = ((ci + 1) * nin) // nch
                        while pi + oi < target:
                            # out units gated until the previous tile's AV
                            # accumulators are fully evacuated (same-engine
                            # emission order == execution order on the DVE).
                            out_ok = (oi < no_ and ci >= 2 and not any(
                                k[1] == qt - 1 for k in state))
                            if out_ok and (pi >= np_ or oi * np_ <= pi * no_):
                                outs[oi]()
                                oi += 1
                            elif pi < np_:
                                projs[pi]()
                                pi += 1
                            elif out_ok:
                                outs[oi]()
                                oi += 1
                            else:
                                break
                    while pi < np_:
                        projs[pi]()
                        pi += 1
                    while oi < no_:
                        assert not any(k[1] == qt - 1 for k in state)
                        outs[oi]()
                        oi += 1

                for p in pendq:
                    emit_av(p)
                pendq = []
                for u in out_stream(NQT - 1):
                    u()
                assert not state

    nc.finalize()
    return nc


def kernel(x, wq, wk, wv, wo):
    import ml_dtypes
    from concourse import bass_utils

    if os.environ.get("BASS_TRACE"):
        _install_axon_ntff_hook()

    bf = ml_dtypes.bfloat16
    x = np.asarray(x, dtype=np.float32)
    wq = np.asarray(wq, dtype=np.float32)
    wk = np.asarray(wk, dtype=np.float32)
    wv = np.asarray(wv, dtype=np.float32)
    wo = np.asarray(wo, dtype=np.float32)

    # Host prep: weight slicing + rope column permutation + tables.
    perm_l = _rope_perm_local()
    perm = np.concatenate([h * HD + perm_l for h in range(NH)])  # [D]
    scale = 1.0 / np.sqrt(HD)
    wq_p = np.ascontiguousarray(wq[:, perm] * scale)
    wk_p = np.ascontiguousarray(wk[:, perm])
    cos_dup, sin_signed = _rope_tables()
    cos_dup = cos_dup.astype(bf)
    sin_signed = sin_signed.astype(bf)
    kl = np.arange(KC)[:, None]
    ql = np.arange(KC)[None, :]
    tri = np.where(ql >= kl, 1.0, 0.0).astype(bf)  # 0/1 probs mask

    # sel[p_src, g*128 + p_dst] = 1 iff p_src == 32 * (2g + p_dst//64):
    # broadcast head (2g + p_dst//64)'s recip row onto all its 64 dims.
    sel = np.zeros((128, GQ, 128), dtype=np.float32)
    for g in range(GQ):
        for a in range(2):
            sel[32 * (2 * g + a), g, a * HD:(a + 1) * HD] = 1.0
    sel = np.ascontiguousarray(sel.reshape(128, GQ * 128).astype(bf))

    # [D, S] -> [128, (D//128)*S]: partition-major with dense 16KB rows
    xTs = [np.ascontiguousarray(
        x[b].T.astype(bf).reshape(D // 128, 128, S).transpose(1, 0, 2)
        .reshape(128, (D // 128) * S)) for b in range(B)]

    in_maps = []
    for i in range(NCORES):
        b, g = divmod(i, HPC)
        cs = slice(g * DC, (g + 1) * DC)
        in_maps.append({
            "xT": xTs[b],
            "wq": np.ascontiguousarray(wq_p[:, cs].astype(bf)),
            "wk": np.ascontiguousarray(wk_p[:, cs].astype(bf)),
            "wv": np.ascontiguousarray(wv[:, cs].astype(bf)),
            "wo": np.ascontiguousarray(wo[cs, :].astype(bf)),
            "cosd": cos_dup,
            "sind": sin_signed,
            "tri": tri,
            "sel": sel,
            "vone": np.ones((128, (S // KC) * HPC), dtype=bf),
        })

    if "nc" not in _CACHE:
        _CACHE["nc"] = _build_program()
    nc = _CACHE["nc"]

    res = bass_utils.run_bass_kernel_spmd(nc, in_maps, core_ids=list(range(NCORES)))
    _CACHE["last_exec_time_ns"] = res.exec_time_ns
    _CACHE["last_res"] = res

    out = np.empty((B, S, D), dtype=np.float32)
    for b in range(B):
        acc = res.results[b * HPC]["outT"].astype(np.float32)
        for g in range(1, HPC):
            acc += res.results[b * HPC + g]["outT"].astype(np.float32)
        out[b] = acc.T
    return out
